# revision 11
# baseline (speedup 1.0000x reference)
"""Trainium2 Bass kernel for gated short-time-warp + Conv1d (nn_GW_Conv1D).

Reference computation (per batch element b, C=64 channels, T=32768):
  g = tanh(einsum('ct,c->t', x, est_w)) * 0.5            # velocity, |g| <= 0.5
  d = flow(g)    per 256-window (scaling & squaring, 4 iters), |d| <= 0.5
  xw = interp1d(x, p + d)   per window                    # forward warp
  y = conv1d(xw, conv_w, conv_b, k=3, SAME)               # channel mixing
  d_inv = flow(-g); out = interp1d(y, p + d_inv)          # inverse warp

Because |d| < 1, each interpolation is a 3-term elementwise expression:
  xw[t] = x[t] + dn[t]*(x[t-1]-x[t]) + dp[t]*(x[t+1]-x[t])
with dn = relu(-d), dp = relu(d), masked to 0 at window edges.

Sharding: pure data parallelism, batch b -> core b (8 cores).

Layout: conv layout (128 partitions = channel + 64*half, H=16384 time columns
per half).  All elementwise work is fp16 TENSOR_TENSOR on the DVE in 2x_1P
perf mode, which requires 4-byte-aligned unit-stride operands.  The +-1
column shifts of the warp are therefore materialized with DMA shift-copies
(xs/ys tiles) so every DVE op reads at even column offsets.

The coefficient (dn/dp) broadcast across the 64 channel partitions is fused
with the warp multiply: a single SWDGE DMA with a stride-0 read AP and
accum_op=mult multiplies the compact per-window coefficients into the
diff tiles in place (the DMA engines do the fanout AND the multiply).

The warp ADDs are folded into the tensor engine: the conv accumulates both
x and t23 = t2+t3 taps (6 matmuls per 512 cols), and the inverse-warp sum
y + w2 + w3 is an identity-matmul accumulation into the conv PSUM bank,
evacuated by one scalar activation (bias + f32 cast).

The chunk loop is software-pipelined at 2-chunk lag so each engine-queue
FIFO only sees dependencies that are >= 1 chunk old.
"""
import sys

sys.path.insert(0, "/opt/trn_rl_repo")

import numpy as np
from contextlib import ExitStack

import concourse.bass as bass
import concourse.tile as tile
from concourse import bacc, mybir
from concourse.ap import AP
from concourse.bass_interp import get_hw_module
from concourse import bass_utils

F32 = mybir.dt.float32
F16 = mybir.dt.float16
AF = mybir.ActivationFunctionType
ALU = mybir.AluOpType

NCORES = 8
C, T, W = 64, 32768, 256
H = T // 2            # 16384 columns per half (stacked-halves conv layout)
CH = 1024             # pipeline chunk width (= 4 windows)
NCH = H // CH         # 16 chunks
SUB = 512             # conv sub-chunk (one PSUM bank)
FLOW_ITERS = 4
LP = 4                # x16 left pad columns (col = t + LP)
XW = H + 12           # x16 width (pads both sides)
YW = H + 4            # y16 width (col = t + 2)
FW = 16392            # fcoeff width (col = t + 2)


def _flow_iteration(nc, pool, d2):
    """One scaling-and-squaring step on d2 (128, 512) fp16 = [d_fwd | d_inv].
    d2 <- d2 + interp1d(d2, p + d2), per 256-column window."""
    dn = pool.tile([128, 512], F16, tag="fl_dn")
    dp = pool.tile([128, 512], F16, tag="fl_dp")
    nc.scalar.activation(dn[:], d2[:], AF.Relu, scale=-1.0)
    nc.scalar.activation(dp[:], d2[:], AF.Relu)
    # window-edge masking (jnp.clip at borders)
    nc.gpsimd.memset(dn[:, 0:1], 0.0)
    nc.gpsimd.memset(dn[:, 256:257], 0.0)
    nc.gpsimd.memset(dp[:, 255:256], 0.0)
    nc.gpsimd.memset(dp[:, 511:512], 0.0)
    am = pool.tile([128, 512], F16, tag="fl_am")
    nc.vector.tensor_tensor(am[:], dn[:], dp[:], ALU.add)
    nc.vector.tensor_scalar(am[:], am[:], -1.0, 1.0, ALU.mult, ALU.add)
    itp = pool.tile([128, 512], F16, tag="fl_itp")
    tmp = pool.tile([128, 512], F16, tag="fl_tmp")
    nc.vector.tensor_tensor(itp[:], d2[:], am[:], ALU.mult)
    # left-neighbour term (dn masked at window starts -> cross-window leak *0)
    nc.vector.tensor_tensor(tmp[:, 1:512], d2[:, 0:511], dn[:, 1:512], ALU.mult)
    nc.vector.tensor_tensor(itp[:, 1:512], itp[:, 1:512], tmp[:, 1:512], ALU.add)
    # right-neighbour term
    nc.vector.tensor_tensor(tmp[:, 0:511], d2[:, 1:512], dp[:, 0:511], ALU.mult)
    nc.vector.tensor_tensor(itp[:, 0:511], itp[:, 0:511], tmp[:, 0:511], ALU.add)
    nc.vector.tensor_tensor(d2[:], d2[:], itp[:], ALU.add)


def _build_module():
    nc = bacc.Bacc("TRN2", target_bir_lowering=False, debug=False,
                   enable_asserts=False, num_devices=NCORES)
    x = nc.dram_tensor("x", (C, T), F32, kind="ExternalInput").ap()
    ew = nc.dram_tensor("ew", (128, 2), F16, kind="ExternalInput").ap()
    cw = nc.dram_tensor("cw", (128, 384), F16, kind="ExternalInput").ap()
    cb = nc.dram_tensor("cb", (128, 1), F32, kind="ExternalInput").ap()
    ident = nc.dram_tensor("ident", (128, 128), F16, kind="ExternalInput").ap()
    y = nc.dram_tensor("y", (C, T), F32, kind="ExternalOutput").ap()

    x_hc = x.rearrange("c (h t) -> h c t", h=2)          # (2, 64, H)
    y_hc = y.rearrange("c (h t) -> h c t", h=2)

    with tile.TileContext(nc) as tc, ExitStack() as ctx:
        sm = ctx.enter_context(tc.tile_pool(name="sm", bufs=1))
        big = ctx.enter_context(tc.tile_pool(name="big", bufs=1))
        xsp = ctx.enter_context(tc.tile_pool(name="xsp", bufs=3))
        cfp = ctx.enter_context(tc.tile_pool(name="cfp", bufs=3))
        cfip = ctx.enter_context(tc.tile_pool(name="cfip", bufs=3))
        ttp = ctx.enter_context(tc.tile_pool(name="ttp", bufs=3))
        t23p = ctx.enter_context(tc.tile_pool(name="t23p", bufs=3))
        ysp = ctx.enter_context(tc.tile_pool(name="ysp", bufs=3))
        wsp = ctx.enter_context(tc.tile_pool(name="wsp", bufs=3))
        w23p = ctx.enter_context(tc.tile_pool(name="w23p", bufs=3))
        yop = ctx.enter_context(tc.tile_pool(name="yop", bufs=3))
        psE = ctx.enter_context(tc.tile_pool(name="psE", bufs=1, space="PSUM"))
        psC = ctx.enter_context(tc.tile_pool(name="psC", bufs=6, space="PSUM"))

        ew_sb = sm.tile([128, 2], F16, tag="ew")
        nc.sync.dma_start(ew_sb[:], ew)
        cw_sb = sm.tile([128, 384], F16, tag="cw")
        nc.sync.dma_start(cw_sb[:], cw)
        cb_sb = sm.tile([128, 1], F32, tag="cb")
        nc.sync.dma_start(cb_sb[:], cb)
        id_sb = sm.tile([128, 128], F16, tag="ident")
        nc.sync.dma_start(id_sb[:], ident)

        x16 = big.tile([128, XW], F16)      # signal fp16, col = t + LP
        y16 = big.tile([128, YW], F16)      # conv output fp16, col = t + 2
        fcf = big.tile([8, FW], F16)        # flat coeffs, col = t + 2
        # rows: 0,1 = dnF(h0,h1)  2,3 = dpF  4,5 = dnI  6,7 = dpI
        t23_15 = big.tile([128, CH + 4], F16)   # chunk 15's t23 (long-lived)

        nc.gpsimd.memset(x16[:, 0:LP], 0.0)
        nc.gpsimd.memset(x16[:, H + LP:XW], 0.0)
        nc.gpsimd.memset(y16[:, 0:2], 0.0)
        nc.gpsimd.memset(y16[:, H + 2:YW], 0.0)
        nc.gpsimd.memset(fcf[:, 0:2], 0.0)
        nc.gpsimd.memset(fcf[:, FW - 6:FW], 0.0)

        # ---------------- Phase 1: load + cast (DMA) + einsum g -------------
        # einsum via x-stationary matmuls: out partition = time-within-128,
        # psum column chosen so the two XBAR transposes land directly in
        # window layout: col(jj, h) = (jj>>1) + 128*(jj&1) + 64*h.
        g_ps = psE.tile([128, 256], F32, tag="gps")
        g_v = g_ps.rearrange("q (a b) -> q a b", b=64)
        for k in range(NCH):
            a = k * CH
            # casting DMA: HBM f32 -> SBUF fp16 (SWDGE sprays all engines)
            nc.gpsimd.dma_start(x16[:, LP + a:LP + a + CH],
                                x_hc[:, :, a:a + CH], single_packet=True)
            for j in range(CH // 128):
                jj = (CH // 128) * k + j
                a0, b0 = 2 * (jj & 1), jj >> 1
                nc.tensor.matmul(g_v[:, a0:a0 + 2, b0:b0 + 1],
                                 x16[:, LP + 128 * jj:LP + 128 * jj + 128],
                                 ew_sb[:], start=True, stop=True)
        # cross-half conv halo columns of x16:
        # half0 t=H <- half1 t=0 ; half1 t=-1 <- half0 t=H-1
        nc.sync.dma_start(x16[0:64, H + LP:H + LP + 1], x16[64:128, LP:LP + 1])
        nc.sync.dma_start(x16[64:128, LP - 1:LP], x16[0:64, H + LP - 1:H + LP])

        # ---------------- Phase 2: g -> window layout, flow, coeffs ---------
        g16 = sm.tile([128, 256], F16, tag="g16")
        nc.scalar.copy(g16[:], g_ps[:])
        # gw16[f=64h+l, w] = g(h, 256l + w): two direct XBAR transposes
        gw16 = sm.tile([128, 256], F16, tag="gw16")
        nc.sync.dma_start_transpose(gw16[:, 0:128], g16[:, 0:128])
        nc.sync.dma_start_transpose(gw16[:, 128:256], g16[:, 128:256])
        gth = sm.tile([128, 256], F16, tag="gth")
        nc.scalar.activation(gth[:], gw16[:], AF.Tanh)
        d2 = sm.tile([128, 512], F16, tag="d2")           # [d_fwd | d_inv]
        nc.vector.tensor_scalar_mul(d2[:, 0:256], gth[:], 0.5 / 16.0)
        nc.vector.tensor_scalar_mul(d2[:, 256:512], gth[:], -0.5 / 16.0)
        for _ in range(FLOW_ITERS):
            _flow_iteration(nc, sm, d2)
        dn2 = sm.tile([128, 512], F16, tag="cf_dn")
        dp2 = sm.tile([128, 512], F16, tag="cf_dp")
        nc.scalar.activation(dn2[:], d2[:], AF.Relu, scale=-1.0)
        nc.scalar.activation(dp2[:], d2[:], AF.Relu)
        nc.gpsimd.memset(dn2[:, 0:1], 0.0)
        nc.gpsimd.memset(dn2[:, 256:257], 0.0)
        nc.gpsimd.memset(dp2[:, 255:256], 0.0)
        nc.gpsimd.memset(dp2[:, 511:512], 0.0)
        # flatten window rows -> one row per (coeff, half): (64,256) -> (1,16384)
        for r, (srcv, off) in enumerate(
                ((dn2, 0), (dp2, 0), (dn2, 256), (dp2, 256))):
            for h in (0, 1):
                nc.sync.dma_start(fcf[2 * r + h:2 * r + h + 1, 2:2 + H],
                                  srcv[64 * h:64 * h + 64, off:off + 256])

        # ---------------- Phase 3: pipelined chunks -------------------------
        t23s = {}

        def tau1(k):
            """Shift-copy, diffs, coefficient mult-DMAs for chunk k."""
            a = k * CH
            xs = xsp.tile([128, 2 * (CH + 4)], F16, tag="xs")
            xsv = xs[:].rearrange("p (s n) -> p s n", s=2)
            # seg 0: x[t-1] (src col t-1+LP = a+1+j), seg 1: x[t+1] (a+3+j)
            base = x16[:, a + 1:a + 2]
            src = AP(base.tensor, base.offset,
                     list(base.ap)[:1] + [(2, 2), (1, CH + 4)])
            nc.sync.dma_start(xsv, src)
            # broadcast compact coeffs across channel partitions (stride-0 read)
            cf = cfp.tile([128, 2 * (CH + 4)], F16, tag="cf")
            for half, r in ((0, 0), (1, 2)):   # dnF rows 0:2, dpF rows 2:4
                dst = cf[:, half * (CH + 4):(half + 1) * (CH + 4)]
                srcc = fcf[r:r + 2, a:a + CH + 4].unsqueeze(1)
                srcc = srcc.to_broadcast([2, 64, CH + 4])
                nc.gpsimd.dma_start(dst, srcc, single_packet=True)
            tt = ttp.tile([128, 2 * (CH + 4)], F16, tag="tt")
            nc.vector.tensor_tensor(tt[:, 0:CH + 4], xs[:, 0:CH + 4],
                                    x16[:, a + 2:a + 2 + CH + 4], ALU.subtract)
            nc.vector.tensor_tensor(tt[:, CH + 4:], xs[:, CH + 4:],
                                    x16[:, a + 2:a + 2 + CH + 4], ALU.subtract)
            nc.vector.tensor_tensor(tt[:, 0:CH + 4], tt[:, 0:CH + 4],
                                    cf[:, 0:CH + 4], ALU.mult)
            nc.vector.tensor_tensor(tt[:, CH + 4:], tt[:, CH + 4:],
                                    cf[:, CH + 4:], ALU.mult)
            return tt

        def tau2(k, tt):
            """t23 = t2 + t3 for chunk k."""
            if k == NCH - 1:
                t23 = t23_15
            else:
                t23 = t23p.tile([128, CH + 4], F16, tag="t23")
            nc.vector.tensor_tensor(t23[:], tt[:, 0:CH + 4], tt[:, CH + 4:],
                                    ALU.add)
            t23s[k] = t23

        def conv(k):
            """6 matmuls per 512-col sub: 3 taps x {x16, t23}."""
            a = k * CH
            t23 = t23s.pop(k)
            banks = []
            for s in range(CH // SUB):
                pc = psC.tile([128, SUB], F32, tag="pc")
                o = a + SUB * s
                for e in range(3):
                    nc.tensor.matmul(
                        pc[:], cw_sb[:, 128 * e:128 * e + 128],
                        x16[:, o + e + LP - 1:o + e + LP - 1 + SUB],
                        start=(e == 0), stop=False)
                for e in range(3):
                    nc.tensor.matmul(
                        pc[:], cw_sb[:, 128 * e:128 * e + 128],
                        t23[:, SUB * s + e + 1:SUB * s + e + 1 + SUB],
                        start=False, stop=(e == 2))
                banks.append(pc)
            return banks

        def acts(k, banks):
            a = k * CH
            for s in range(CH // SUB):
                nc.scalar.copy(y16[:, 2 + a + SUB * s:2 + a + SUB * (s + 1)],
                               banks[s][:])

        def omega1(j):
            """y shift-copy, inverse diffs, inverse coefficient mult-DMAs."""
            b = j * CH
            ys = ysp.tile([128, 2 * CH], F16, tag="ys")
            ysv = ys[:].rearrange("p (s n) -> p s n", s=2)
            base = y16[:, b + 1:b + 2]
            src = AP(base.tensor, base.offset,
                     list(base.ap)[:1] + [(2, 2), (1, CH)])
            nc.sync.dma_start(ysv, src)
            cf = cfip.tile([128, 2 * CH], F16, tag="cfi")
            for half, r in ((0, 4), (1, 6)):   # dnI rows 4:6, dpI rows 6:8
                dst = cf[:, half * CH:(half + 1) * CH]
                srcc = fcf[r:r + 2, b + 2:b + 2 + CH].unsqueeze(1)
                srcc = srcc.to_broadcast([2, 64, CH])
                nc.gpsimd.dma_start(dst, srcc, single_packet=True)
            ws = wsp.tile([128, 2 * CH], F16, tag="ws")
            nc.vector.tensor_tensor(ws[:, 0:CH], ys[:, 0:CH],
                                    y16[:, b + 2:b + 2 + CH], ALU.subtract)
            nc.vector.tensor_tensor(ws[:, CH:], ys[:, CH:],
                                    y16[:, b + 2:b + 2 + CH], ALU.subtract)
            nc.vector.tensor_tensor(ws[:, 0:CH], ws[:, 0:CH],
                                    cf[:, 0:CH], ALU.mult)
            nc.vector.tensor_tensor(ws[:, CH:], ws[:, CH:],
                                    cf[:, CH:], ALU.mult)
            return ws

        def omega2(j, ws, banks):
            """w23 sum, identity-matmul accumulate, final act, store."""
            b = j * CH
            w23 = w23p.tile([128, CH], F16, tag="w23")
            nc.vector.tensor_tensor(w23[:], ws[:, 0:CH], ws[:, CH:], ALU.add)
            yo = yop.tile([128, CH], F32, tag="yo")
            for s in range(CH // SUB):
                nc.tensor.matmul(banks[s][:], id_sb[:],
                                 w23[:, SUB * s:SUB * s + SUB],
                                 start=False, stop=True, skip_group_check=True)
                nc.scalar.activation(yo[:, SUB * s:SUB * s + SUB], banks[s][:],
                                     AF.Identity, bias=cb_sb[:])
            nc.sync.dma_start(y_hc[:, :, b:b + CH], yo[:])

        # prologue: chunk 15 (cross-half halo source) then chunk 0
        tt15 = tau1(NCH - 1)
        tau2(NCH - 1, tt15)
        tt0 = tau1(0)
        tau2(0, tt0)
        # cross-half halos on the t23 tiles (conv taps that cross t=H):
        # half0 t=H <- half1 t=0 ; half1 t=-1 <- half0 t=H-1
        nc.sync.dma_start(t23s[NCH - 1][0:64, CH + 2:CH + 3],
                          t23s[0][64:128, 2:3])
        nc.sync.dma_start(t23s[0][64:128, 1:2],
                          t23s[NCH - 1][0:64, CH + 1:CH + 2])
        tt1 = tau1(1)
        tau2(1, tt1)

        banks = {}
        wss = {}
        tts = {1: tt1}
        for k in range(NCH):
            banks[k] = conv(k)
            acts(k, banks[k])
            if 2 <= k + 2 <= NCH - 2:
                tts[k + 2] = tau1(k + 2)
            if 2 <= k + 1 <= NCH - 2:
                tau2(k + 1, tts.pop(k + 1))
            if k >= 1:
                wss[k - 1] = omega1(k - 1)
            if k >= 2:
                omega2(k - 2, wss.pop(k - 2), banks.pop(k - 2))
        wss[NCH - 1] = omega1(NCH - 1)
        omega2(NCH - 2, wss.pop(NCH - 2), banks.pop(NCH - 2))
        omega2(NCH - 1, wss.pop(NCH - 1), banks.pop(NCH - 1))

    nc.compile()
    return nc


def _host_params(est_w, conv_w, conv_b):
    ew = np.zeros((128, 2), np.float16)
    ew[:64, 0] = est_w
    ew[64:, 1] = est_w
    cw = np.zeros((128, 384), np.float16)
    for j in range(3):
        blk = conv_w[:, :, j].T.astype(np.float16)   # (in, out)
        cw[0:64, j * 128:j * 128 + 64] = blk
        cw[64:128, j * 128 + 64:j * 128 + 128] = blk
    cb = np.concatenate([conv_b, conv_b]).astype(np.float32)[:, None]
    ident = np.eye(128, dtype=np.float16)
    return ew, cw, cb, ident


_COMPILED = None


def _get_compiled():
    global _COMPILED
    if _COMPILED is None:
        nc = _build_module()
        nc.m = get_hw_module(nc.m)
        _COMPILED = nc
    return _COMPILED


def kernel(signal, est_w, conv_w, conv_b, _trace=False, _trace_kwargs=None):
    nc = _get_compiled()
    ew, cw, cb, ident = _host_params(np.asarray(est_w, np.float32),
                                     np.asarray(conv_w, np.float32),
                                     np.asarray(conv_b, np.float32))
    signal = np.ascontiguousarray(np.asarray(signal, np.float32))
    in_maps = [{"x": signal[b], "ew": ew, "cw": cw, "cb": cb, "ident": ident}
               for b in range(NCORES)]
    res = bass_utils.run_bass_kernel_spmd(
        nc, in_maps, core_ids=list(range(NCORES)), trace=_trace,
        **(_trace_kwargs or {}))
    out = np.stack([r["y"] for r in res.results], axis=0)
    if _trace:
        return out, res
    return out


# revision 22
# speedup vs baseline: 1.1229x; 1.1229x over previous
"""Trainium2 Bass kernel for gated short-time-warp + Conv1d (nn_GW_Conv1D).

Reference computation (per batch element b, C=64 channels, T=32768):
  g = tanh(einsum('ct,c->t', x, est_w)) * 0.5            # velocity, |g| <= 0.5
  d = flow(g)    per 256-window (scaling & squaring, 4 iters), |d| <= 0.5
  xw = interp1d(x, p + d)   per window                    # forward warp
  y = conv1d(xw, conv_w, conv_b, k=3, SAME)               # channel mixing
  d_inv = flow(-g); out = interp1d(y, p + d_inv)          # inverse warp

Because |d| < 1, each interpolation is a 3-term elementwise expression:
  xw[t] = x[t] + dn[t]*(x[t-1]-x[t]) + dp[t]*(x[t+1]-x[t])
with dn = relu(-d), dp = relu(d), masked to 0 at window edges.

Sharding: pure data parallelism, batch b -> core b (8 cores).

Layout: conv layout (128 partitions = channel + 64*half, H=16384 time columns
per half).  All elementwise work is fp16 TENSOR_TENSOR on the DVE in 2x_1P
perf mode (2 elem/cycle/lane), which requires 4-byte-aligned unit-stride
operands.  The +-1 column shifts of the warp are served by ONE full-width
shifted copy (x16m / y16m, built with cheap contiguous DMAs off the critical
path): x[t-1] and x[t+1] both land at even column offsets in the shifted
tensor, so every diff is aligned.

Coefficients are broadcast across the 64 channel partitions with stride-0
read-AP DMAs from a compact (128, 1028) per-chunk layout (fcf2: partition
8k+2c+h holds coeff class c, half h, chunk k).

The warp ADDs are folded into the tensor engine: the conv accumulates x, t2
and t3 taps (9 matmuls per 512 cols, shifts absorbed by the moving-operand
address), and the inverse-warp sum y + w2 + w3 is an identity-matmul
accumulation into the conv PSUM bank, evacuated by one scalar activation
(bias + f32 cast).

The chunk loop is software-pipelined (forward side 2 chunks ahead, inverse
side 2 behind) so each engine-queue FIFO only sees dependencies >= 1 chunk
old, hiding DMA completion latency.
"""
import sys

sys.path.insert(0, "/opt/trn_rl_repo")

import numpy as np
from contextlib import ExitStack

import concourse.bass as bass
import concourse.tile as tile
from concourse import bacc, mybir
from concourse.bass_interp import get_hw_module
from concourse import bass_utils

F32 = mybir.dt.float32
F16 = mybir.dt.float16
AF = mybir.ActivationFunctionType
ALU = mybir.AluOpType

NCORES = 8
C, T, W = 64, 32768, 256
H = T // 2            # 16384 columns per half (stacked-halves conv layout)
CH = 1024             # pipeline chunk width (= 4 windows)
NCH = H // CH         # 16 chunks
SUB = 512             # conv sub-chunk (one PSUM bank)
FLOW_ITERS = 4
LP = 4                # x16 left pad columns (col = t + LP)
XW = H + 8            # x16/x16m width
YW = H + 4            # y16 width (col = t + 2)
YMW = H + 4           # y16m width (y16m[c] = y16[c-1])
CB = CH + 4           # per-chunk coefficient/product tile width
FB = 8196             # flat coeff block width (col = t - 8192*m + 2)


def _flow_iteration(nc, pool, d2):
    """One scaling-and-squaring step on d2 (128, 512) fp16 = [d_fwd | d_inv].
    d2 <- d2 + interp1d(d2, p + d2), per 256-column window."""
    dn = pool.tile([128, 512], F16, tag="fl_dn")
    dp = pool.tile([128, 512], F16, tag="fl_dp")
    nc.scalar.activation(dn[:], d2[:], AF.Relu, scale=-1.0)
    nc.scalar.activation(dp[:], d2[:], AF.Relu)
    # window-edge masking (jnp.clip at borders)
    nc.gpsimd.memset(dn[:, 0:1], 0.0)
    nc.gpsimd.memset(dn[:, 256:257], 0.0)
    nc.gpsimd.memset(dp[:, 255:256], 0.0)
    nc.gpsimd.memset(dp[:, 511:512], 0.0)
    am = pool.tile([128, 512], F16, tag="fl_am")
    nc.vector.tensor_tensor(am[:], dn[:], dp[:], ALU.add)
    nc.vector.tensor_scalar(am[:], am[:], -1.0, 1.0, ALU.mult, ALU.add)
    itp = pool.tile([128, 512], F16, tag="fl_itp")
    tmp = pool.tile([128, 512], F16, tag="fl_tmp")
    nc.vector.tensor_tensor(itp[:], d2[:], am[:], ALU.mult)
    # left-neighbour term (dn masked at window starts -> cross-window leak *0)
    nc.vector.tensor_tensor(tmp[:, 1:512], d2[:, 0:511], dn[:, 1:512], ALU.mult)
    nc.vector.tensor_tensor(itp[:, 1:512], itp[:, 1:512], tmp[:, 1:512], ALU.add)
    # right-neighbour term
    nc.vector.tensor_tensor(tmp[:, 0:511], d2[:, 1:512], dp[:, 0:511], ALU.mult)
    nc.vector.tensor_tensor(itp[:, 0:511], itp[:, 0:511], tmp[:, 0:511], ALU.add)
    nc.vector.tensor_tensor(d2[:], d2[:], itp[:], ALU.add)


def _build_module():
    nc = bacc.Bacc("TRN2", target_bir_lowering=False, debug=False,
                   enable_asserts=False, num_devices=NCORES)
    x = nc.dram_tensor("x", (C, T), F32, kind="ExternalInput").ap()
    ew = nc.dram_tensor("ew", (128, 2), F16, kind="ExternalInput").ap()
    cw = nc.dram_tensor("cw", (128, 384), F16, kind="ExternalInput").ap()
    cb = nc.dram_tensor("cb", (128, 1), F32, kind="ExternalInput").ap()
    ident = nc.dram_tensor("ident", (128, 128), F16, kind="ExternalInput").ap()
    y = nc.dram_tensor("y", (C, T), F32, kind="ExternalOutput").ap()

    x_hc = x.rearrange("c (h t) -> h c t", h=2)          # (2, 64, H)
    y_hc = y.rearrange("c (h t) -> h c t", h=2)

    with tile.TileContext(nc) as tc, ExitStack() as ctx:
        sm = ctx.enter_context(tc.tile_pool(name="sm", bufs=1))
        big = ctx.enter_context(tc.tile_pool(name="big", bufs=1))
        cfp = ctx.enter_context(tc.tile_pool(name="cfp", bufs=3))
        ttp = ctx.enter_context(tc.tile_pool(name="ttp", bufs=3))
        wsp = ctx.enter_context(tc.tile_pool(name="wsp", bufs=2))
        yop = ctx.enter_context(tc.tile_pool(name="yop", bufs=2))
        psC = ctx.enter_context(tc.tile_pool(name="psC", bufs=8, space="PSUM"))

        ew_sb = sm.tile([128, 2], F16, tag="ew")
        nc.sync.dma_start(ew_sb[:], ew)
        cw_sb = sm.tile([128, 384], F16, tag="cw")
        nc.sync.dma_start(cw_sb[:], cw)
        cb_sb = sm.tile([128, 1], F32, tag="cb")
        nc.sync.dma_start(cb_sb[:], cb)
        id_sb = sm.tile([128, 128], F16, tag="ident")
        nc.sync.dma_start(id_sb[:], ident)

        x16 = big.tile([128, XW], F16)     # signal fp16, col = t + LP
        x16m = big.tile([128, XW], F16)    # x16 shifted right by one column
        y16 = big.tile([128, YW], F16)     # conv output fp16, col = t + 2
        y16m = big.tile([128, YMW], F16)   # y16 shifted right by one column
        # flat coeffs split into two 8192-column time blocks m:
        # row 8m + (0,1)=dnF(h0,h1) (2,3)=dpF (4,5)=dnI (6,7)=dpI,
        # col j <-> t = 8192*m - 2 + j.  rows 16..23 are spare (the race
        # checker's conservative reach for the row-14/15 broadcast pair).
        fcf = big.tile([24, FB], F16)

        nc.gpsimd.memset(x16[:, 0:LP], 0.0)
        nc.gpsimd.memset(x16[:, H + LP:XW], 0.0)
        nc.gpsimd.memset(x16m[:, 0:LP + 1], 0.0)
        nc.gpsimd.memset(y16[:, 0:2], 0.0)
        nc.gpsimd.memset(y16[:, H + 2:YW], 0.0)
        nc.gpsimd.memset(y16m[:, 0:3], 0.0)
        nc.gpsimd.memset(y16m[:, H + 3:YMW], 0.0)
        # chunk 15 is processed first; its w2 reads one column of chunk 14's
        # y16m segment (window-boundary masked, but must be initialized)
        nc.gpsimd.memset(y16m[:, 3 + (NCH - 1) * CH - 1:3 + (NCH - 1) * CH],
                         0.0)
        nc.gpsimd.memset(fcf[0:24, 0:2], 0.0)
        nc.gpsimd.memset(fcf[0:24, FB - 2:FB], 0.0)

        # ---------------- Phase 1: load + cast (DMA) + einsum g -------------
        # einsum via x-stationary matmuls: out partition = time-within-128,
        # psum column chosen so the two XBAR transposes land directly in
        # window layout: col(jj, h) = (jj>>1) + 128*(jj&1) + 64*h.
        g_ps = psC.tile([128, 256], F32, tag="pc")
        g_v = g_ps.rearrange("q (a b) -> q a b", b=64)
        for k in range(NCH):
            a = k * CH
            # casting DMA: HBM f32 -> SBUF fp16 (SWDGE sprays all engines)
            nc.gpsimd.dma_start(x16[:, LP + a:LP + a + CH],
                                x_hc[:, :, a:a + CH], single_packet=True)
            # shifted copy (contiguous rows, off the critical path)
            nc.sync.dma_start(x16m[:, LP + a + 1:LP + a + CH + 1],
                              x16[:, LP + a:LP + a + CH])
            for j in range(CH // 128):
                jj = (CH // 128) * k + j
                a0, b0 = 2 * (jj & 1), jj >> 1
                nc.tensor.matmul(g_v[:, a0:a0 + 2, b0:b0 + 1],
                                 x16[:, LP + 128 * jj:LP + 128 * jj + 128],
                                 ew_sb[:], start=True, stop=True)
        # x16m tail (t = H .. H+2 region, all zero pads anyway)
        nc.sync.dma_start(x16m[:, LP + H + 1:XW], x16[:, LP + H:XW - 1])
        # cross-half conv halo columns of x16:
        # half0 t=H <- half1 t=0 ; half1 t=-1 <- half0 t=H-1
        nc.sync.dma_start(x16[0:64, H + LP:H + LP + 1], x16[64:128, LP:LP + 1])
        nc.sync.dma_start(x16[64:128, LP - 1:LP], x16[0:64, H + LP - 1:H + LP])

        # ---------------- Phase 2: g -> window layout, flow, coeffs ---------
        g16 = sm.tile([128, 256], F16, tag="g16")
        nc.scalar.copy(g16[:], g_ps[:])
        # gw16[f=64h+l, w] = g(h, 256l + w): two direct XBAR transposes
        gw16 = sm.tile([128, 256], F16, tag="gw16")
        nc.sync.dma_start_transpose(gw16[:, 0:128], g16[:, 0:128])
        nc.sync.dma_start_transpose(gw16[:, 128:256], g16[:, 128:256])
        gth = sm.tile([128, 256], F16, tag="gth")
        nc.scalar.activation(gth[:], gw16[:], AF.Tanh)
        d2 = sm.tile([128, 512], F16, tag="d2")           # [d_fwd | d_inv]
        nc.vector.tensor_scalar_mul(d2[:, 0:256], gth[:], 0.5 / 16.0)
        nc.vector.tensor_scalar_mul(d2[:, 256:512], gth[:], -0.5 / 16.0)
        for _ in range(FLOW_ITERS):
            _flow_iteration(nc, sm, d2)
        dn2 = sm.tile([128, 512], F16, tag="fl_dn")
        dp2 = sm.tile([128, 512], F16, tag="fl_dp")
        nc.scalar.activation(dn2[:], d2[:], AF.Relu, scale=-1.0)
        nc.scalar.activation(dp2[:], d2[:], AF.Relu)
        nc.gpsimd.memset(dn2[:, 0:1], 0.0)
        nc.gpsimd.memset(dn2[:, 256:257], 0.0)
        nc.gpsimd.memset(dp2[:, 255:256], 0.0)
        nc.gpsimd.memset(dp2[:, 511:512], 0.0)
        # pack into fcf2: partition 8k + 2c + h <- coeff class c, half h,
        # chunk k (columns 2..CB-2 = t in [k*CH-2, k*CH+CH+2)).
        srcs = ((dn2, 0), (dp2, 0), (dn2, 256), (dp2, 256))
        for c, (srcv, off) in enumerate(srcs):
            for h in (0, 1):
                for m in (0, 1):
                    rr = 8 * m + 2 * c + h
                    nc.sync.dma_start(
                        fcf[rr:rr + 1, 2:2 + H // 2],
                        srcv[64 * h + 32 * m:64 * h + 32 * m + 32,
                             off:off + 256])
        for rr in range(8):
            # block 1 left edge (t = 8190, 8191) = block 0 cols 8192:8194
            nc.scalar.dma_start(fcf[8 + rr:9 + rr, 0:2],
                                fcf[rr:rr + 1, 2 + H // 2 - 2:2 + H // 2])
            # block 0 right edge (t = 8192, 8193) = block 1 cols 2:4
            nc.scalar.dma_start(fcf[rr:rr + 1, FB - 2:FB],
                                fcf[8 + rr:9 + rr, 2:4])

        # ---------------- Phase 3: pipelined chunks -------------------------
        # per-chunk tiles are indexed by t = k*CH - 2 + j  (j in [0, CB))
        tts = {}

        def bcast_f(k):
            a = k * CH
            cf = cfp.tile([128, 2 * CB], F16, tag="cf")
            m, al = divmod(a, H // 2)
            for half in (0, 1):   # dnF = rows 0:2, dpF = rows 2:4
                srcc = fcf[8 * m + 2 * half:8 * m + 2 * half + 2, al:al + CB]
                srcc = srcc.unsqueeze(1).to_broadcast([2, 64, CB])
                nc.gpsimd.dma_start(cf[:, half * CB:(half + 1) * CB], srcc,
                                    single_packet=True)
            return cf

        def tdiff(k):
            a = k * CH
            tt = ttp.tile([128, 2 * CB], F16, tag="tt")
            # t2[j] = x[t-1] - x[t]   (x16m col t+LP, x16 col t+LP-2.. wait)
            nc.vector.tensor_tensor(tt[:, 0:CB],
                                    x16m[:, a + 2:a + 2 + CB],
                                    x16[:, a + 2:a + 2 + CB], ALU.subtract)
            # t3[j] = x[t+1] - x[t]   (x16m col t+6)
            nc.vector.tensor_tensor(tt[:, CB:],
                                    x16m[:, a + 4:a + 4 + CB],
                                    x16[:, a + 2:a + 2 + CB], ALU.subtract)
            return tt

        def tmul(k, tt, cf):
            nc.vector.tensor_tensor(tt[:, 0:CB], tt[:, 0:CB], cf[:, 0:CB],
                                    ALU.mult)
            nc.vector.tensor_tensor(tt[:, CB:], tt[:, CB:], cf[:, CB:],
                                    ALU.mult)

        def conv(k):
            a = k * CH
            tt = tts.pop(k)
            banks = []
            for s in range(CH // SUB):
                pc = psC.tile([128, SUB], F32, tag="pc")
                o = a + SUB * s
                for e in range(3):
                    nc.tensor.matmul(
                        pc[:], cw_sb[:, 128 * e:128 * e + 128],
                        x16[:, o + e + LP - 1:o + e + LP - 1 + SUB],
                        start=(e == 0), stop=False)
                for e in range(3):
                    nc.tensor.matmul(
                        pc[:], cw_sb[:, 128 * e:128 * e + 128],
                        tt[:, SUB * s + e + 1:SUB * s + e + 1 + SUB],
                        start=False, stop=False)
                for e in range(3):
                    nc.tensor.matmul(
                        pc[:], cw_sb[:, 128 * e:128 * e + 128],
                        tt[:, CB + SUB * s + e + 1:CB + SUB * s + e + 1 + SUB],
                        start=False, stop=(e == 2))
                banks.append(pc)
            return banks

        def acts(k, banks):
            a = k * CH
            for s in range(CH // SUB):
                nc.scalar.copy(y16[:, 2 + a + SUB * s:2 + a + SUB * (s + 1)],
                               banks[s][:])
            # shifted copy for the inverse diffs
            nc.sync.dma_start(y16m[:, 3 + a:3 + a + CH],
                              y16[:, 2 + a:2 + a + CH])
            # y16m cols >= H+3 stay at their memset zeros (window-end masked)

        def bcast_i(j):
            cf = cfp.tile([128, 2 * CB], F16, tag="cf")
            m, bl = divmod(j * CH, H // 2)
            for half in (0, 1):   # dnI = rows 4:6, dpI = rows 6:8
                srcc = fcf[8 * m + 4 + 2 * half:8 * m + 6 + 2 * half,
                           bl:bl + CB]
                srcc = srcc.unsqueeze(1).to_broadcast([2, 64, CB])
                nc.gpsimd.dma_start(cf[:, half * CB:(half + 1) * CB], srcc,
                                    single_packet=True)
            return cf

        def omega1(j, cf):
            """Inverse diffs + coefficient mults (w2 | w3)."""
            b = j * CH
            ws = wsp.tile([128, 2 * CH], F16, tag="ws")
            # w2[t] = y[t-1] - y[t]: y16m col t+2, y16 col t+2
            nc.vector.tensor_tensor(ws[:, 0:CH], y16m[:, b + 2:b + 2 + CH],
                                    y16[:, b + 2:b + 2 + CH], ALU.subtract)
            # w3[t] = y[t+1] - y[t]: y16m col t+4
            nc.vector.tensor_tensor(ws[:, CH:], y16m[:, b + 4:b + 4 + CH],
                                    y16[:, b + 2:b + 2 + CH], ALU.subtract)
            nc.vector.tensor_tensor(ws[:, 0:CH], ws[:, 0:CH],
                                    cf[:, 2:2 + CH], ALU.mult)
            nc.vector.tensor_tensor(ws[:, CH:], ws[:, CH:],
                                    cf[:, CB + 2:CB + 2 + CH], ALU.mult)
            return ws

        def omega2(j, ws, banks):
            """Identity-matmul accumulate, final act, store."""
            b = j * CH
            yo = yop.tile([128, CH], F32, tag="yo")
            for s in range(CH // SUB):
                nc.tensor.matmul(banks[s][:], id_sb[:],
                                 ws[:, SUB * s:SUB * s + SUB],
                                 start=False, stop=False,
                                 skip_group_check=True)
                nc.tensor.matmul(banks[s][:], id_sb[:],
                                 ws[:, CH + SUB * s:CH + SUB * s + SUB],
                                 start=False, stop=True,
                                 skip_group_check=True)
                nc.scalar.activation(yo[:, SUB * s:SUB * s + SUB], banks[s][:],
                                     AF.Identity, bias=cb_sb[:])
            nc.sync.dma_start(y_hc[:, :, b:b + CH], yo[:])

        # prologue: forward products for chunks 15 and 0 (chunk 15 is
        # processed first: it is the cross-half halo source for chunk 0
        # and vice versa)
        SEQ = [NCH - 1] + list(range(NCH - 1))
        cf15 = bcast_f(SEQ[0])
        t15 = tdiff(SEQ[0])
        tmul(SEQ[0], t15, cf15)
        tts[SEQ[0]] = t15
        cf0 = bcast_f(SEQ[1])
        t0 = tdiff(SEQ[1])
        tmul(SEQ[1], t0, cf0)
        tts[SEQ[1]] = t0
        # cross-half halos on the t2|t3 tiles (conv taps crossing t=H):
        # half0 t=H <- half1 t=0 ; half1 t=-1 <- half0 t=H-1
        for seg in (0, CB):
            nc.sync.dma_start(t15[0:64, seg + CH + 2:seg + CH + 3],
                              t0[64:128, seg + 2:seg + 3])
            nc.sync.dma_start(t0[64:128, seg + 1:seg + 2],
                              t15[0:64, seg + CH + 1:seg + CH + 2])

        banks = {}
        wss = {}
        cfs = {}
        for i in range(NCH):
            k = SEQ[i]
            banks[k] = conv(k)
            acts(k, banks[k])
            if i + 2 < NCH:
                k2 = SEQ[i + 2]
                cfs[k2] = (bcast_f(k2), tdiff(k2))
            if i + 1 < NCH and SEQ[i + 1] in cfs:
                k1 = SEQ[i + 1]
                cf_, tt_ = cfs.pop(k1)
                tmul(k1, tt_, cf_)
                tts[k1] = tt_
            if i >= 2:
                j = SEQ[i - 2]
                cfi = bcast_i(j)
                wss[j] = omega1(j, cfi)
            if i >= 3:
                j = SEQ[i - 3]
                omega2(j, wss.pop(j), banks.pop(j))
        for i in (NCH - 2, NCH - 1):
            j = SEQ[i]
            cfi = bcast_i(j)
            wss[j] = omega1(j, cfi)
        for i in (NCH - 3, NCH - 2, NCH - 1):
            j = SEQ[i]
            omega2(j, wss.pop(j), banks.pop(j))

    nc.compile()
    return nc


def _host_params(est_w, conv_w, conv_b):
    ew = np.zeros((128, 2), np.float16)
    ew[:64, 0] = est_w
    ew[64:, 1] = est_w
    cw = np.zeros((128, 384), np.float16)
    for j in range(3):
        blk = conv_w[:, :, j].T.astype(np.float16)   # (in, out)
        cw[0:64, j * 128:j * 128 + 64] = blk
        cw[64:128, j * 128 + 64:j * 128 + 128] = blk
    cb = np.concatenate([conv_b, conv_b]).astype(np.float32)[:, None]
    ident = np.eye(128, dtype=np.float16)
    return ew, cw, cb, ident


_COMPILED = None


def _get_compiled():
    global _COMPILED
    if _COMPILED is None:
        nc = _build_module()
        nc.m = get_hw_module(nc.m)
        _COMPILED = nc
    return _COMPILED


def kernel(signal, est_w, conv_w, conv_b, _trace=False, _trace_kwargs=None):
    nc = _get_compiled()
    ew, cw, cb, ident = _host_params(np.asarray(est_w, np.float32),
                                     np.asarray(conv_w, np.float32),
                                     np.asarray(conv_b, np.float32))
    signal = np.ascontiguousarray(np.asarray(signal, np.float32))
    in_maps = [{"x": signal[b], "ew": ew, "cw": cw, "cb": cb, "ident": ident}
               for b in range(NCORES)]
    res = bass_utils.run_bass_kernel_spmd(
        nc, in_maps, core_ids=list(range(NCORES)), trace=_trace,
        **(_trace_kwargs or {}))
    out = np.stack([r["y"] for r in res.results], axis=0)
    if _trace:
        return out, res
    return out
